# revision 46
# baseline (speedup 1.0000x reference)
"""GroupedTernaryLinear Trainium2 kernel (Bass/Tile, 8-core SPMD).

Computation (matches the jax reference):
  x:      [2, 4096, 4096] f32   -> flatten to [8192, 4096] tokens
  weight: [4096, 1024]    f32
  1. xn = rms_norm(x) over last dim (eps = f32 eps)
  2. w_bf = bf16(weight); per flat 64-chunk: scale = bf16(mean|w_bf|) (clipped),
     q = clip(round(w_bf/scale), -1, 1)  ->  wq = q*scale  (exact in bf16)
  3. out[t, g*1024+o] = sum_i xn[t, g*1024+i] * wq[g*1024+o, i]   (4 groups)

Kernel strategy (v2):
  - Shard 8192 tokens across 8 cores (1024 each); weight replicated.
  - rms_norm folded AFTER the matmul (linearity): out = fac[t] * (x @ wq^T).
  - Weight path: SWDGE cast-DMA loads w as bf16; per [128o,1024i] tile:
      red = sum|w| per 64-chunk        (GPSIMD tensor_reduce)
      s   = bf16(max(red/64, 1e-8))    (DVE tiny tensor_scalar)
      s_full = broadcast s over 64     (ACT copy from stride-0 view)
      g = (w*(1/THR)) is_gt s_full     (DVE scalar_tensor_tensor, bf16 2x)
      l = (w*(-1/THR)) is_gt s_full    (DVE scalar_tensor_tensor)
      d = g - l;  wq = d * s_full      (DVE tensor_tensor)
    then PE-transpose wq into resident wqT [i, g, k, o] (bf16).
  - x path: HWDGE f32 loads; ACT square+accum -> rms fac; PE-transpose raw
    f32 x -> bf16 xT via psum evac cast; grouped matmul; fac folded into
    the PSUM->SBUF output evacuation; quarter-block stores.
  - PE queue interleaving: x-transposes of blocks 0-3 first, then per group
    {8 w-tile transpose quads + MMs of blocks 0-3}, then blocks 4-7 pure.
"""

import os
import sys

sys.path.insert(0, "/opt/trn_rl_repo")

import numpy as np

import concourse.bass as bass
import concourse.mybir as mybir
import concourse.tile as tile
from concourse import bacc
from concourse.bass_utils import run_bass_kernel_spmd
from concourse.masks import make_identity

F32 = mybir.dt.float32
BF16 = mybir.dt.bfloat16
AF = mybir.ActivationFunctionType
ALU = mybir.AluOpType


def _register_ternary_op():
    """Register a fused custom-DVE op computing the whole ternary quantize:

        out = ((in0 > in1*imm2) - (in0 < -(in1*imm2))) * in1

    (in0 = bf16 weight tile, in1 = per-chunk scale broadcast, imm2 = THR).
    One 1x DVE pass replaces the g/l/d/wq 4-pass sequence. The registry in
    concourse.dve_ops is in-process module state shared by codegen, the
    per-NEFF table generator and CoreSim, so appending here is sufficient.
    """
    import concourse.dve_ops as dve_ops
    from concourse.dve_spec import Spec, Src0, Src1, C0, Zero, lower, _has_src1
    from concourse.dve_uop import DveOpSpec

    name = "TERNARY_QUANT_ANT"
    for op in dve_ops.OPS:
        if op.name == name:
            return op

    _t = Src1 * C0
    spec = Spec(
        body=((Src0 > _t) - ((Zero - Src0) > _t)) * Src1,
        reference=lambda in0, in1, s0, s1, imm2: (
            (
                (in0.astype(np.float32) > in1 * s0).astype(np.float32)
                - (-in0.astype(np.float32) > in1 * s0).astype(np.float32)
            )
            * in1
        ).astype(np.float32),
    )
    row = max(dve_ops._SUB_OPCODE_FOR_NAME.values()) + 1
    assert row < 0x20
    dve_ops._SUB_OPCODE_FOR_NAME[name] = row
    shas = {}
    for ver in ("v3", "v4"):
        try:
            uops = lower(spec, ver=ver)
            shas[ver] = DveOpSpec(
                name=name, opcode=row, uops=uops, rd1_en=_has_src1(spec)
            ).sha(ver)
        except Exception:
            pass
    assert shas, "ternary custom-DVE spec failed to lower"
    op = dve_ops.DveOp(name, spec, subdim=False, uops_sha=shas)
    dve_ops.OPS.append(op)
    dve_ops.CUSTOM_DVE_SPECS[name] = spec
    return op


TERNARY_OP = _register_ternary_op()

N_CORES = 8
T = 1024          # tokens per core
D = 4096          # feature dim (= 4 groups * 1024)
G = 4             # groups
GI = 1024         # group input dim
GO = 1024         # group output dim
KC = D // 128     # 32 k-chunks of 128 over the full feature dim
GK = GI // 128    # 8 k-chunks per group
TB = T // 128     # 8 token blocks per core
EPS = 1.1920929e-07          # np.finfo(np.float32).eps
THR = 0.5009765625           # bf16 round-to-nearest-even threshold for |r|>0.5
INV_THR = 1.0 / THR

LAST_EXEC_NS = None
LAST_RESULTS = None


def _build():
    nc = bacc.Bacc("TRN2", target_bir_lowering=False, debug=False)
    x_ap = nc.dram_tensor("x", [T, D], F32, kind="ExternalInput").ap()
    w_ap = nc.dram_tensor("weight", [D, GI], F32, kind="ExternalInput").ap()
    out_ap = nc.dram_tensor("out", [T, D], F32, kind="ExternalOutput").ap()

    with tile.TileContext(nc) as tc:
        _body(tc, nc, out_ap, x_ap, w_ap)

    nc.compile()
    return nc


def _body(tc, nc, out_ap, x_ap, w_ap):
    with (
        tc.tile_pool(name="consts", bufs=1) as consts,
        tc.tile_pool(name="wqt", bufs=1) as wqt_pool,
        tc.tile_pool(name="win", bufs=3) as win_pool,
        tc.tile_pool(name="wq", bufs=4) as wq_pool,
        tc.tile_pool(name="tiny", bufs=4) as tiny_pool,
        tc.tile_pool(name="xin", bufs=2) as xin_pool,
        tc.tile_pool(name="xbf", bufs=4) as xbf_pool,
        tc.tile_pool(name="junk", bufs=1) as junk_pool,
        tc.tile_pool(name="xtp", bufs=TB) as xtp_pool,
        tc.tile_pool(name="stats", bufs=TB) as stats_pool,
        tc.tile_pool(name="outsb", bufs=3) as out_pool,
        tc.tile_pool(name="ps_tp", bufs=4, space="PSUM") as ps_tp,
        tc.tile_pool(name="ps_mm", bufs=2, space="PSUM") as ps_mm,
    ):
        ident_f = consts.tile([128, 128], F32, name="ident_f")
        make_identity(nc, ident_f[:])
        ident_b = consts.tile([128, 128], BF16, name="ident_b")
        make_identity(nc, ident_b[:])
        eps_t = consts.tile([128, 1], F32, name="eps_t")
        nc.vector.memset(eps_t[:], EPS)

        # Resident transposed-quantized weight: [i(128), g, k, o] bf16
        wqT = wqt_pool.tile([128, G, GK, GO], BF16, name="wqT")

        # ---- weight loads: SWDGE cast f32->bf16, 2 o-tiles per DMA --------
        w_view = w_ap.rearrange("(j p) i -> p j i", p=128)   # [128, 32, 1024]
        wbf_tiles = {}

        def load_pack(pk):
            """cast-load w o-tiles (2*pk, 2*pk+1) as one 1 MB SWDGE DMA."""
            if pk in wbf_tiles or pk >= D // 256:
                return
            wbf = win_pool.tile([128, 2, GI], BF16, name="wbf")
            nc.gpsimd.dma_start(wbf[:], w_view[:, 2 * pk:2 * pk + 2, :])
            wbf_tiles[pk] = wbf

        # ---- per-block helpers -------------------------------------------
        HD = D // 2        # half-block free size (1 MB f32 reads, cast bf16)

        def x_quad(xb, xT, lc, co, pick):
            """one 4-transpose quad + evac; lc = column in xb, co = in xT."""
            def emit():
                xps = ps_tp.tile([128, 4, 128], BF16, name="qps")
                for j in range(4):
                    cc = lc + j
                    nc.tensor.transpose(
                        xps[:, j, :], xb[:, cc * 128:(cc + 1) * 128],
                        ident_b[:],
                    )
                if pick % 2 == 0:
                    nc.scalar.copy(xT[:, co:co + 4, :], xps[:])
                else:
                    nc.vector.tensor_copy(xT[:, co:co + 4, :], xps[:])
            return emit

        def x_block_load(b, inline_quads=True):
            """load block b (f32 halves), DVE-cast bf16, ACT square-accum.
            Returns (xT, sshs, quad thunks); quads emitted inline if asked."""
            xT = xtp_pool.tile([128, KC, 128], BF16, name="xT")
            sshs = []
            quads = []
            for h in range(2):
                xt = xin_pool.tile([128, HD], F32, name="xt")
                nc.sync.dma_start(
                    xt[:], x_ap[b * 128:(b + 1) * 128, h * HD:(h + 1) * HD],
                )
                xb = xbf_pool.tile([128, HD], BF16, name="xb")
                nc.vector.tensor_copy(xb[:], xt[:])
                for c0 in range(0, KC // 2, 4):
                    q = x_quad(xb, xT, c0, h * (KC // 2) + c0, b + c0 // 4)
                    if inline_quads:
                        q()
                    else:
                        quads.append(q)
                junk = junk_pool.tile([128, HD], BF16, name="junk")
                ssh = stats_pool.tile([128, 1], F32, name=f"ss{h}")
                nc.scalar.activation(junk[:], xb[:], AF.Square, accum_out=ssh[:])
                sshs.append(ssh)
            return xT, sshs, quads

        def x_block_stats(sshs):
            """finish the rms factor (emitted after the chunk's w-tiles)."""
            ss = stats_pool.tile([128, 1], F32, name="ss")
            nc.gpsimd.tensor_add(ss[:], sshs[0][:], sshs[1][:])
            sq = stats_pool.tile([128, 1], F32, name="sq")
            nc.scalar.activation(sq[:], ss[:], AF.Sqrt, bias=eps_t[:], scale=1.0 / D)
            fac = stats_pool.tile([128, 1], F32, name="fac")
            nc.vector.reciprocal(fac[:], sq[:])
            return fac

        def mm_group(b, g, xT, fac, evac_pick, pop_quad=None):
            """matmuls + fac-folded evac + store for (block b, group g).
            pop_quad() emits one pending transpose quad; called between
            accumulation steps to keep the HAM busy-window fed without long
            MM-free stretches (foreign PE ops inside the accumulation group
            are HW-legal; skip the sim's contiguity check)."""
            pm = ps_mm.tile([128, 2, 512], F32, name="pm")
            for k in range(GK):
                lhsT = xT[:, g * GK + k, :]
                nc.tensor.matmul(
                    pm[:, 0, :], lhsT, wqT[:, g, k, 0:512],
                    start=(k == 0), stop=(k == GK - 1),
                    skip_group_check=True,
                )
                nc.tensor.matmul(
                    pm[:, 1, :], lhsT, wqT[:, g, k, 512:1024],
                    start=(k == 0), stop=(k == GK - 1),
                    skip_group_check=True,
                )
                if pop_quad is not None:
                    pop_quad()
            ob = out_pool.tile([128, GO], F32, name="ob")
            nc.scalar.activation(ob[:, 0:512], pm[:, 0, :], AF.Copy,
                                 scale=fac[:])
            nc.scalar.activation(ob[:, 512:1024], pm[:, 1, :], AF.Copy,
                                 scale=fac[:])
            nc.scalar.dma_start(
                out_ap[b * 128:(b + 1) * 128, g * GO:(g + 1) * GO], ob[:],
            )

        def w_tile_quant(ow):
            """quantize o-tile ow ([128 o, 1024 i]) -> ternary wq (DVE only)."""
            if ow % 2 == 0:
                load_pack(ow // 2 + 2)                       # prefetch ahead
            wbf = wbf_tiles[ow // 2][:, ow % 2, :]           # [128, 1024] bf16

            red = tiny_pool.tile([128, 16], F32, name="red")
            wbf_v = wbf.rearrange("p (c v) -> p c v", v=64)
            nc.vector.tensor_reduce(
                red[:], wbf_v, axis=mybir.AxisListType.X, op=ALU.add,
                apply_absolute_value=True,
            )
            s_bf = tiny_pool.tile([128, 16], BF16, name="s_bf")
            nc.vector.tensor_scalar(
                s_bf[:], red[:], 1.0 / 64.0, 1e-8, ALU.mult, ALU.max,
            )
            s_b = s_bf[:].unsqueeze(2).broadcast_to((128, 16, 64))
            wq = wq_pool.tile([128, GI], BF16, name="wq")
            nc.vector._custom_dve(
                TERNARY_OP, out=wq[:], in0=wbf, in1=s_b, s0=THR,
            )
            return wq

        def w_quad(ow, wq, k0):
            """one 4-transpose quad + evac of wq columns [k0, k0+4)."""
            g, o_off = ow // 8, (ow % 8) * 128

            def emit():
                wps = ps_tp.tile([128, 4, 128], BF16, name="qps")
                for j in range(4):
                    kk = k0 + j
                    nc.tensor.transpose(
                        wps[:, j, :], wq[:, kk * 128:(kk + 1) * 128],
                        ident_b[:],
                    )
                nc.scalar.copy(
                    wqT[:, g, k0:k0 + 4, o_off:o_off + 128], wps[:],
                )
            return emit

        def w_chunk_quant(g):
            """quantize chunk g's 8 tiles; return their transpose quads."""
            quads = []
            for ow in range(g * 8, (g + 1) * 8):
                wq = w_tile_quant(ow)
                quads.append(w_quad(ow, wq, 0))
                quads.append(w_quad(ow, wq, 4))
            return quads

        # ---- interleaved schedule ----------------------------------------
        # Prelude: blocks 0-3 load + transpose inline, chunk-0 quantize +
        # transpose. Then per phase g: chunk g+1's quantize runs on DVE
        # while the PE runs phase-g MMs with chunk g+1's w-quads and the
        # newly loading blocks' x-quads threaded between accumulation steps
        # (keeps the HAM busy-window fed -> MMs stay at 2.4 GHz).
        load_pack(0)
        load_pack(1)
        load_pack(2)
        load_pack(3)

        facs = {}
        xTs = {}
        PHASE_BLOCKS = [[4, 5], [6, 7], [], []]
        PHASE_MMS = [
            [(0, 0), (1, 0), (2, 0), (3, 0)],
            [(0, 1), (1, 1), (2, 1), (3, 1), (4, 0), (5, 0)],
            [(0, 2), (1, 2), (2, 2), (3, 2), (4, 1), (5, 1), (6, 0), (7, 0)],
            [(0, 3), (1, 3), (2, 3), (3, 3), (4, 2), (5, 2), (6, 1), (7, 1)],
        ]
        TAIL_MMS = [(4, 3), (5, 3), (6, 2), (7, 2), (6, 3), (7, 3)]

        # prelude
        c0_quads = w_chunk_quant(0)
        pend = []
        for b in range(4):
            xTs[b], sshs, _ = x_block_load(b, inline_quads=True)
            pend.append((b, sshs))
            for q in c0_quads[b * 4:(b + 1) * 4]:
                q()
        for b, sshs in pend:
            facs[b] = x_block_stats(sshs)

        evac_pick = 0
        for g in range(G):
            quads = []
            if g + 1 < G:
                wquads = w_chunk_quant(g + 1)
            else:
                wquads = []
            xquads = []
            pend = []
            for b in PHASE_BLOCKS[g]:
                xTs[b], sshs, qs = x_block_load(b, inline_quads=False)
                pend.append((b, sshs))
                xquads.extend(qs)
            # alternate w/x quads (matches their production order)
            wi = xi = 0
            while wi < len(wquads) or xi < len(xquads):
                if xi < len(xquads):
                    quads.append(xquads[xi]); xi += 1
                if wi < len(wquads):
                    quads.append(wquads[wi]); wi += 1
                if wi < len(wquads):
                    quads.append(wquads[wi]); wi += 1
            for b, sshs in pend:
                facs[b] = x_block_stats(sshs)

            qiter = iter(quads)

            def pop_quad():
                q = next(qiter, None)
                if q is not None:
                    q()

            for n, (b, gg) in enumerate(PHASE_MMS[g]):
                # first group of each phase: let DVE build a quantize lead
                pq = pop_quad if n > 0 else None
                mm_group(b, gg, xTs[b], facs[b], evac_pick, pq)
                evac_pick += 1
            for q in qiter:      # leftovers
                q()
        for b, gg in TAIL_MMS:
            mm_group(b, gg, xTs[b], facs[b], evac_pick)
            evac_pick += 1


_NC_CACHE = None


def _ensure_ntff_hook():
    """Install the antenv.axon_hooks shim + ctypes NTFF hook if missing."""
    import types

    try:
        from antenv.axon_hooks import get_axon_ntff_profile_hook  # noqa: F401
        return
    except ImportError:
        pass
    import antenv

    mod = types.ModuleType("antenv.axon_hooks")
    mod._hook = None
    mod.set_axon_ntff_profile_hook = lambda h: setattr(mod, "_hook", h)
    mod.get_axon_ntff_profile_hook = lambda: mod._hook
    sys.modules["antenv.axon_hooks"] = mod
    antenv.axon_hooks = mod
    try:
        if "/root/.axon_site" not in sys.path:
            sys.path.insert(0, "/root/.axon_site")
        from trn_agent_boot.trn_boot import _ntff_profile_via_ctypes

        mod.set_axon_ntff_profile_hook(
            _ntff_profile_via_ctypes("/opt/axon/libaxon_pjrt.so")
        )
    except Exception:
        pass


def kernel(x: np.ndarray, weight: np.ndarray) -> np.ndarray:
    global LAST_EXEC_NS, LAST_RESULTS, _NC_CACHE
    x = np.ascontiguousarray(np.asarray(x, dtype=np.float32))
    weight = np.ascontiguousarray(np.asarray(weight, dtype=np.float32))
    lead = x.shape[:-1]
    xf = x.reshape(-1, D)
    assert xf.shape[0] == N_CORES * T, xf.shape

    if _NC_CACHE is None:
        _NC_CACHE = _build()
    nc = _NC_CACHE

    in_maps = [
        {"x": xf[i * T:(i + 1) * T], "weight": weight} for i in range(N_CORES)
    ]
    trace = bool(int(os.environ.get("CCK_TRACE", "0")))
    kw = {}
    if trace:
        _ensure_ntff_hook()
        tdir = os.environ.get("CCK_TRACE_DIR")
        if tdir:
            os.makedirs(tdir, exist_ok=True)
            kw["tmpdir"] = tdir
    res = run_bass_kernel_spmd(nc, in_maps, list(range(N_CORES)), trace=trace, **kw)
    LAST_EXEC_NS = res.exec_time_ns
    LAST_RESULTS = res
    out = np.concatenate([res.results[i]["out"] for i in range(N_CORES)], axis=0)
    return out.reshape(*lead, D).astype(np.float32, copy=False)


if __name__ == "__main__":
    rng = np.random.default_rng(0)
    x = rng.standard_normal((2, 4096, 4096), dtype=np.float32)
    w = (rng.standard_normal((4096, 1024), dtype=np.float32) * 0.02).astype(np.float32)
    o = kernel(x, w)
    print(o.shape, o.dtype, LAST_EXEC_NS)
